# revision 58
# baseline (speedup 1.0000x reference)
"""Trainium2 Bass kernel for leave-one-out Nadaraya-Watson regression
(nn_Net_72877005078649) — fast-Gauss-transform formulation, v23.

Per output channel o this is 1D Gaussian kernel regression; the kernel
factorizes through a G=12 grid (a = b = h/sqrt(2), trapezoid aliasing
~1e-4):  K_h(x,z) ~= kappa * sum_g exp(-(c_g-x)^2/h^2) exp(-(z-c_g)^2/h^2)

v23 design notes (vs v2 baseline at ~41.5us):
 - per-core input roll: core c's train data is rotated so its own 512
   queries are train chunk 0 — the query-side Eq is just cols 0:512 of
   pair-0's train-side Gaussian table ET0. No separate query chain.
 - fused Gaussian via ACT Derivative_Erf = (2/sqrt(pi))exp(-u^2) with
   scale=1/h, per-partition bias=-c_g/h: one ACT op turns the W2rep
   projection (PSUM) straight into kernel factors. (pi/4 folds into
   kappa since both query and train factors carry 2/sqrt(pi).)
 - DMA: descriptor count rules ring throughput (~18ns/descriptor), so
   ship W1T|W2rep|all-h0-chunks as ONE [64, 2304] bf16 tensor into
   partition rows 0:64 of a [128, 2304] tile (64 descriptors of 4.5KB)
   and the h1 twin into rows 64:128 on a second ring. W1T is
   duplicated in both so each partition-half matmuls with a
   base-aligned lhsT (PE runs the two K=64 quadrant matmuls
   concurrently). Y tables as two [128, 2048] tensors. 5 input DMAs.
 - num tables via DVE scalar_tensor_tensor (bf16 mult + accum_out);
   den tables ride the DErf accum_out. relu split ACT/DVE per stage.
   Train side runs as 5 stages [512,1024,1024,1024,512]: stage 0 is
   h0-only (just the queries — starts on half the data, ~1us earlier)
   and stage 4 is h1-only, so the final serial relu->W2->DErf->STT
   drain into the finalize is one short 512-col chain.
 - finalize: fin_j [128q,20] = matmul(lhsT=ET0[:, j*128:+128] bf16,
   rhs=AA bf16) gives num|den in query-partition layout; single
   subtract against (Y|ones), fast reciprocal; output ships as a
   contiguous [128, 40] DMA and the host un-interleaves (a strided
   512x40B scatter delayed the final drain barrier by ~1us).
 - diagonal: train_X == x by construction, K_ii == 1 exactly:
   out = (num - Y_d)/(den - 1).

Sharding: queries split across 8 cores (512/core); train replicated.
"""

import numpy as np

N = 4096
D = 64
HID = 128
O = 10
NCORES = 8
BQ = N // NCORES
G = 12
GRID_LO = -6.5
GRID_HI = 6.5
NPAIR = 4               # train chunk pairs, 1024 cols each
TW = 256 + 2048         # packed tX width: W1T | W2rep-half | 4 h-chunks

_cache = {}


def _host_consts(h: float):
    c = np.linspace(GRID_LO, GRID_HI, G).astype(np.float32)
    delta = float(c[1] - c[0])
    kappa = 2.0 * delta / (np.sqrt(2.0 * np.pi) * h)
    # ET is computed via Derivative_Erf = (2/sqrt(pi))*exp(-u^2); both the
    # query and train factors carry 2/sqrt(pi), so fold (pi/4) into kappa.
    kap = kappa * np.pi / 4.0
    cbase = np.zeros((128, 22), np.float32)
    for p in range(128):
        cbase[p, 0] = c[min(p // O, G - 1)]
        cbase[p, 21] = -c[min(p // O, G - 1)] / h   # DErf bias: -c_g/h
    for p in range(G * O):
        cbase[p, 1 + p % O] = kap            # num mask
        cbase[p, 11 + p % O] = kap           # den mask
    return cbase


def _host_yext(Y, c):
    # yext[p, j*20+e] = Y[c*512 + j*128 + p, e] for e<10, 1.0 for e>=10;
    # lets the finalize subtract num-Y and den-1 in a single tensor_tensor.
    yext = np.ones((128, 4, 2 * O), np.float32)
    yext[:, :, 0:O] = Y[c * BQ:(c + 1) * BQ].reshape(4, 128, O).transpose(1, 0, 2)
    return yext.reshape(128, 4 * 2 * O)


def _build(h: float):
    import concourse.bass as bass
    import concourse.bacc as bacc
    import concourse.tile as tile
    from concourse import mybir

    f32 = mybir.dt.float32
    bf16 = mybir.dt.bfloat16
    AF = mybir.ActivationFunctionType
    ALU = mybir.AluOpType

    inv_h = 1.0 / h

    nc = bacc.Bacc("TRN2", target_bir_lowering=False, debug=False, num_devices=1)
    tXad = nc.dram_tensor("tXa", [64, TW], bf16, kind="ExternalInput").ap()
    tXbd = nc.dram_tensor("tXb", [64, TW], bf16, kind="ExternalInput").ap()
    Yrd = [nc.dram_tensor(f"Yr{q}", [128, 1024], bf16, kind="ExternalInput").ap()
           for q in range(NPAIR)]
    cpd = nc.dram_tensor("cpack", [128, 102], f32, kind="ExternalInput").ap()
    out = nc.dram_tensor("out", [128, 4 * O], f32, kind="ExternalOutput").ap()

    with tile.TileContext(nc) as tc:
        with (
            tc.tile_pool(name="S", bufs=1) as S,
            tc.tile_pool(name="W", bufs=2) as W,
            tc.tile_pool(name="PS", bufs=1, space="PSUM") as PS,
        ):
            # ---- warmup tiles: wmm feeds PE warmup (vector memset only);
            # warm triggers the ACT table load early
            wmm = S.tile([1, 16], f32)
            nc.vector.memset(wmm, 0.0)
            warm = S.tile([1, 16], f32)
            nc.vector.memset(warm, 0.0)
            nc.scalar.activation(out=warm, in_=warm, func=AF.Derivative_Erf)

            # ---- input DMAs: per-pair slices; Tile tracks sub-tile ranges,
            # so each consumer waits only for its own slice. The gpsimd
            # SWDGE ring is ~4x faster (~190GB/s) than the HWDGE rings
            # (~50GB/s), and the scalar ring is further blocked early by the
            # ACT table-load DMAs — so: critical pair-0/1 chunks + Y tables
            # on gpsimd, weight heads + late chunks on sync, nothing on
            # scalar.
            T = S.tile([128, TW], bf16, name="T")
            Yr = [S.tile([128, 1024], bf16, name=f"Yr{q}") for q in range(NPAIR)]
            cp = S.tile([128, 102], f32)

            # sync: a-side (head, queries, then later a-chunks + consts);
            # gpsimd: b-side + Y tables interleaved by deadline
            nc.sync.dma_start(out=T[0:64, 0:256], in_=tXad[:, 0:256])
            nc.gpsimd.dma_start(out=T[64:128, 0:256], in_=tXbd[:, 0:256])
            nc.sync.dma_start(out=T[0:64, 256:768], in_=tXad[:, 256:768])
            nc.gpsimd.dma_start(out=T[64:128, 256:768], in_=tXbd[:, 256:768])
            nc.sync.dma_start(out=T[0:64, 768:1280], in_=tXad[:, 768:1280])
            nc.gpsimd.dma_start(out=Yr[0], in_=Yrd[0])
            nc.gpsimd.dma_start(out=T[64:128, 768:1280], in_=tXbd[:, 768:1280])
            nc.sync.dma_start(out=T[0:64, 1280:1792], in_=tXad[:, 1280:1792])
            nc.sync.dma_start(out=cp, in_=cpd)
            nc.gpsimd.dma_start(out=T[64:128, 1280:1792], in_=tXbd[:, 1280:1792])
            nc.sync.dma_start(out=T[0:64, 1792:2304], in_=tXad[:, 1792:2304])
            nc.gpsimd.dma_start(out=Yr[1], in_=Yrd[1])
            nc.gpsimd.dma_start(out=T[64:128, 1792:2304], in_=tXbd[:, 1792:2304])
            nc.gpsimd.dma_start(out=Yr[2], in_=Yrd[2])
            nc.gpsimd.dma_start(out=Yr[3], in_=Yrd[3])

            w1a = T[0:64, 0:128]
            w1b = T[64:128, 0:128]
            w2r = T[:, 128:256]
            kmask2 = cp[:, 1:21]
            ncq = cp[:, 21:22]
            yext = cp[:, 22:102]

            nparts = S.tile([128, NPAIR + 1], f32)
            dparts = S.tile([128, NPAIR + 1], f32)
            parts = S.tile([128, 2], f32)
            AA = S.tile([128, 2 * O], bf16)
            ET0 = S.tile([128, 1024], bf16)
            nall = S.tile([128, 4 * 2 * O], f32)
            rsb = S.tile([128, 4 * O], f32)
            osb = S.tile([128, 4 * O], f32)

            # ---- PE warmup: dummy matmuls during the DMA wait trigger the
            # HAM clock boost before the real pipeline starts
            wps = PS.tile([128, 1024], f32, tag="hps", bufs=2, name="wps")
            for _ in range(12):
                nc.tensor.matmul(wps[0:16, 0:16], lhsT=wmm[0:1, 0:16],
                                 rhs=wmm[0:1, 0:16], start=True, stop=True)

            # ---- 5 train stages; s0 = queries only (h0-half, starts on
            # half the data), s1-s3 = full 1024-wide pairs, s4 = h1-only 512
            # so the final serial drain into the finalize is short.
            SA = [(0, 512), (512, 1024), (1024, 1536), (1536, 2048), None]
            SB = [None, (0, 512), (512, 1024), (1024, 1536), (1536, 2048)]
            SWD = [512, 1024, 1024, 1024, 512]
            RELU_ACT = [False, True, False, False, True]
            hps_t = []

            def w1_stage(s):
                hps = PS.tile([128, 1024], f32, tag="hps", bufs=2,
                              name=f"hps{s}")
                hps_t.append(hps)
                # h0 -> cols [0:512] (bank0), h1 -> [512:1024] (bank1) so
                # the concurrent quadrant MMs hit different PSUM banks;
                # an h1-only stage writes [0:512] alone.
                if SA[s] is not None:
                    a0, a1 = SA[s]
                    nc.tensor.matmul(hps[:, 0:512], lhsT=w1a,
                                     rhs=T[0:64, 256 + a0:256 + a1],
                                     start=True, stop=True)
                if SB[s] is not None:
                    b0, b1 = SB[s]
                    dst = slice(512, 1024) if SA[s] is not None else slice(0, 512)
                    nc.tensor.matmul(hps[:, dst], lhsT=w1b,
                                     rhs=T[64:128, 256 + b0:256 + b1],
                                     start=True, stop=True)

            w1_stage(0)
            for s in range(5):
                w = SWD[s]
                h1 = W.tile([128, 1024], bf16, tag="h1", bufs=4)
                if RELU_ACT[s]:
                    nc.scalar.activation(out=h1[:, 0:w], in_=hps_t[s][:, 0:w],
                                         func=AF.Relu)
                else:
                    nc.vector.tensor_scalar(out=h1[:, 0:w],
                                            in0=hps_t[s][:, 0:w],
                                            scalar1=0.0,
                                            scalar2=None, op0=ALU.max)
                if s + 1 < 5:
                    w1_stage(s + 1)
                xr = PS.tile([128, 1024], f32, tag="xr", bufs=2, name=f"xr{s}")
                nc.tensor.matmul(xr[:, 0:512], lhsT=w2r, rhs=h1[:, 0:512],
                                 start=True, stop=True)
                if w > 512:
                    nc.tensor.matmul(xr[:, 512:1024], lhsT=w2r,
                                     rhs=h1[:, 512:1024],
                                     start=True, stop=True)
                # fused Gaussian: DErf(xr/h - c_g/h) = 2/sqrt(pi) exp(-s(xr-c)^2)
                ET = ET0 if s == 0 else W.tile([128, 1024], bf16, tag="ET",
                                               bufs=4)
                scr = W.tile([128, 1024], bf16, tag="scr", bufs=4)
                Yin = (Yr[0][:, 0:512] if s == 0 else
                       Yr[0][:, 512:1024] if s == 4 else Yr[s])
                nc.scalar.activation(out=ET[:, 0:w], in_=xr[:, 0:w],
                                     func=AF.Derivative_Erf,
                                     bias=ncq, scale=inv_h,
                                     accum_out=dparts[:, s:s + 1])
                nc.vector.scalar_tensor_tensor(
                    out=scr[:, 0:w], in0=ET[:, 0:w], scalar=1.0,
                    in1=Yin, op0=ALU.bypass, op1=ALU.mult,
                    accum_out=nparts[:, s:s + 1])

            # ---- tables -> AA (bf16 for the bf16 fin matmuls) ----
            nc.vector.tensor_reduce(out=parts[:, 0:1], in_=nparts,
                                    axis=mybir.AxisListType.X, op=ALU.add)
            nc.vector.tensor_reduce(out=parts[:, 1:2], in_=dparts,
                                    axis=mybir.AxisListType.X, op=ALU.add)
            PP = parts.ap[0][0]
            parts_b = bass.AP(tensor=parts.tensor, offset=parts.offset,
                              ap=[[PP, 128], [1, 2], [0, O]])
            nc.vector.tensor_tensor(out=AA.rearrange("p (k e) -> p k e", e=O),
                                    in0=parts_b,
                                    in1=kmask2.rearrange("p (k e) -> p k e", e=O),
                                    op=ALU.mult)

            # ---- query contraction directly into query-partition layout
            fin = PS.tile([128, 4 * 2 * O], f32, tag="xr", bufs=2, name="fin")
            for j in range(4):
                nc.tensor.matmul(fin[:, j * 20:(j + 1) * 20],
                                 lhsT=ET0[:, j * 128:(j + 1) * 128], rhs=AA,
                                 start=True, stop=True)

            # single subtract: num-Y and den-1 at once (yext = Y | ones)
            nc.vector.tensor_tensor(out=nall, in0=fin, in1=yext,
                                    op=ALU.subtract)
            NP = nall.ap[0][0]
            numv = bass.AP(tensor=nall.tensor, offset=nall.offset,
                           ap=[[NP, 128], [2 * O, 4], [1, O]])
            denv = bass.AP(tensor=nall.tensor, offset=nall.offset + O,
                           ap=[[NP, 128], [2 * O, 4], [1, O]])
            nc.vector.reciprocal_approx_fast(
                rsb.rearrange("p (j o) -> p j o", o=O), denv)
            nc.vector.tensor_tensor(
                out=osb.rearrange("p (j o) -> p j o", o=O),
                in0=numv, in1=rsb.rearrange("p (j o) -> p j o", o=O),
                op=ALU.mult)
            nc.sync.dma_start(out=out, in_=osb)

    nc.compile()
    return nc


def build_in_maps(x, train_X, Y, W1, W2, h):
    import jax.numpy as jnp

    def bf(a):
        return np.asarray(jnp.asarray(np.ascontiguousarray(a),
                                      dtype=jnp.bfloat16))

    cbase = _host_consts(float(h))
    x = np.ascontiguousarray(x, dtype=np.float32)
    train_X = np.ascontiguousarray(train_X, dtype=np.float32)
    Y = np.ascontiguousarray(Y, dtype=np.float32)
    W1 = np.ascontiguousarray(W1, dtype=np.float32)
    W2 = np.ascontiguousarray(W2, dtype=np.float32)

    pmod = np.arange(128) % O
    W2rep = W2[pmod].T                  # [128k, 128p]; W2rep[k,p] = W2[p%10,k]

    idx = np.arange(N)
    in_maps = []
    for c in range(NCORES):
        n_list = (idx + c * BQ) % N     # core's own queries first
        Xp = train_X[n_list]            # [N, 64]
        Yp = Y[n_list][:, pmod]         # [N, 128]
        tXa = np.concatenate([W1.T, W2rep[0:64, :], Xp[0:2048].T], axis=1)
        tXb = np.concatenate([W1.T, W2rep[64:128, :], Xp[2048:4096].T], axis=1)
        # pair q columns: [h0 = points q*512:(q+1)*512 | h1 = 2048+same]
        cpack = np.zeros((128, 102), np.float32)
        cpack[:, 0:22] = cbase
        cpack[:, 22:102] = _host_yext(Y, c)
        m = {"tXa": bf(tXa), "tXb": bf(tXb), "cpack": cpack}
        # Yr0 = [stage0 (queries, a-pts 0:512) | stage4 (b-pts 1536:2048)];
        # Yr1..Yr3 = stage s: [a-pts s*512:(s+1)*512 | b-pts (s-1)*512:s*512]
        m["Yr0"] = bf(np.concatenate(
            [Yp[0:512].T, Yp[2048 + 1536:2048 + 2048].T], axis=1))
        for s in range(1, 4):
            m[f"Yr{s}"] = bf(np.concatenate(
                [Yp[s * 512:(s + 1) * 512].T,
                 Yp[2048 + (s - 1) * 512:2048 + s * 512].T], axis=1))
        in_maps.append(m)
    return in_maps


def kernel(x, train_X, Y, W1, W2, h):
    import concourse.bass_utils as bass_utils

    hval = float(h)
    key = ("fgt25", hval)
    if key not in _cache:
        _cache[key] = _build(hval)
    nc = _cache[key]

    in_maps = build_in_maps(x, train_X, Y, W1, W2, h)
    res = bass_utils.run_bass_kernel_spmd(nc, in_maps, core_ids=list(range(NCORES)))
    # device out is [128, 4*O] with row p, col j*O+e = query (j*128+p);
    # un-interleave on host
    outs = []
    for c in range(NCORES):
        a = np.asarray(res.results[c]["out"]).reshape(128, 4, O)
        outs.append(np.ascontiguousarray(a.transpose(1, 0, 2)).reshape(BQ, O))
    return np.concatenate(outs, axis=0)


# revision 59
# speedup vs baseline: 1.0232x; 1.0232x over previous
"""Trainium2 Bass kernel for leave-one-out Nadaraya-Watson regression
(nn_Net_72877005078649) — fast-Gauss-transform formulation, v23.

Per output channel o this is 1D Gaussian kernel regression; the kernel
factorizes through a G=12 grid (a = b = h/sqrt(2), trapezoid aliasing
~1e-4):  K_h(x,z) ~= kappa * sum_g exp(-(c_g-x)^2/h^2) exp(-(z-c_g)^2/h^2)

v23 design notes (vs v2 baseline at ~41.5us):
 - per-core input roll: core c's train data is rotated so its own 512
   queries are train chunk 0 — the query-side Eq is just cols 0:512 of
   pair-0's train-side Gaussian table ET0. No separate query chain.
 - fused Gaussian via ACT Derivative_Erf = (2/sqrt(pi))exp(-u^2) with
   scale=1/h, per-partition bias=-c_g/h: one ACT op turns the W2rep
   projection (PSUM) straight into kernel factors. (pi/4 folds into
   kappa since both query and train factors carry 2/sqrt(pi).)
 - DMA: descriptor count rules ring throughput (~18ns/descriptor), so
   ship W1T|W2rep|all-h0-chunks as ONE [64, 2304] bf16 tensor into
   partition rows 0:64 of a [128, 2304] tile (64 descriptors of 4.5KB)
   and the h1 twin into rows 64:128 on a second ring. W1T is
   duplicated in both so each partition-half matmuls with a
   base-aligned lhsT (PE runs the two K=64 quadrant matmuls
   concurrently). Y tables as two [128, 2048] tensors. 5 input DMAs.
 - num tables via DVE scalar_tensor_tensor (bf16 mult + accum_out);
   den tables ride the DErf accum_out. relu split ACT/DVE per stage.
   Train side runs as 5 stages [512,1024,1024,1024,512]: stage 0 is
   h0-only (just the queries — starts on half the data, ~1us earlier)
   and stage 4 is h1-only, so the final serial relu->W2->DErf->STT
   drain into the finalize is one short 512-col chain.
 - finalize: fin_j [128q,20] = matmul(lhsT=ET0[:, j*128:+128] bf16,
   rhs=AA bf16) gives num|den in query-partition layout; single
   subtract against (Y|ones), fast reciprocal; output ships as a
   contiguous [128, 40] DMA and the host un-interleaves (a strided
   512x40B scatter delayed the final drain barrier by ~1us).
 - diagonal: train_X == x by construction, K_ii == 1 exactly:
   out = (num - Y_d)/(den - 1).

Sharding: queries split across 8 cores (512/core); train replicated.
"""

import numpy as np

N = 4096
D = 64
HID = 128
O = 10
NCORES = 8
BQ = N // NCORES
G = 12
GRID_LO = -6.5
GRID_HI = 6.5
NPAIR = 4               # train chunk pairs, 1024 cols each
TW = 256 + 2048         # packed tX width: W1T | W2rep-half | 4 h-chunks

_cache = {}


def _host_consts(h: float):
    c = np.linspace(GRID_LO, GRID_HI, G).astype(np.float32)
    delta = float(c[1] - c[0])
    kappa = 2.0 * delta / (np.sqrt(2.0 * np.pi) * h)
    # ET is computed via Derivative_Erf = (2/sqrt(pi))*exp(-u^2); both the
    # query and train factors carry 2/sqrt(pi), so fold (pi/4) into kappa.
    kap = kappa * np.pi / 4.0
    cbase = np.zeros((128, 22), np.float32)
    for p in range(128):
        cbase[p, 0] = c[min(p // O, G - 1)]
        cbase[p, 21] = -c[min(p // O, G - 1)] / h   # DErf bias: -c_g/h
    for p in range(G * O):
        cbase[p, 1 + p % O] = kap            # num mask
        cbase[p, 11 + p % O] = kap           # den mask
    return cbase


def _host_yext(Y, c):
    # yext[p, j*20+e] = Y[c*512 + j*128 + p, e] for e<10, 1.0 for e>=10;
    # lets the finalize subtract num-Y and den-1 in a single tensor_tensor.
    yext = np.ones((128, 4, 2 * O), np.float32)
    yext[:, :, 0:O] = Y[c * BQ:(c + 1) * BQ].reshape(4, 128, O).transpose(1, 0, 2)
    return yext.reshape(128, 4 * 2 * O)


def _build(h: float):
    import concourse.bass as bass
    import concourse.bacc as bacc
    import concourse.tile as tile
    from concourse import mybir

    f32 = mybir.dt.float32
    bf16 = mybir.dt.bfloat16
    AF = mybir.ActivationFunctionType
    ALU = mybir.AluOpType

    inv_h = 1.0 / h

    nc = bacc.Bacc("TRN2", target_bir_lowering=False, debug=False, num_devices=1)
    tXad = nc.dram_tensor("tXa", [64, TW], bf16, kind="ExternalInput").ap()
    tXbd = nc.dram_tensor("tXb", [64, TW], bf16, kind="ExternalInput").ap()
    Yrd = [nc.dram_tensor(f"Yr{q}", [128, 1024], bf16, kind="ExternalInput").ap()
           for q in range(NPAIR)]
    cpd = nc.dram_tensor("cpack", [128, 102], f32, kind="ExternalInput").ap()
    out = nc.dram_tensor("out", [128, 4 * O], f32, kind="ExternalOutput").ap()

    with tile.TileContext(nc) as tc:
        with (
            tc.tile_pool(name="S", bufs=1) as S,
            tc.tile_pool(name="W", bufs=2) as W,
            tc.tile_pool(name="PS", bufs=1, space="PSUM") as PS,
        ):
            # ---- warmup tiles: wmm feeds PE warmup (vector memset only);
            # warm triggers the ACT table load early
            wmm = S.tile([1, 16], f32)
            nc.vector.memset(wmm, 0.0)
            warm = S.tile([1, 16], f32)
            nc.vector.memset(warm, 0.0)
            nc.scalar.activation(out=warm, in_=warm, func=AF.Derivative_Erf)

            # ---- input DMAs: per-pair slices; Tile tracks sub-tile ranges,
            # so each consumer waits only for its own slice. The gpsimd
            # SWDGE ring is ~4x faster (~190GB/s) than the HWDGE rings
            # (~50GB/s), and the scalar ring is further blocked early by the
            # ACT table-load DMAs — so: critical pair-0/1 chunks + Y tables
            # on gpsimd, weight heads + late chunks on sync, nothing on
            # scalar.
            T = S.tile([128, TW], bf16, name="T")
            Yr = [S.tile([128, 1024], bf16, name=f"Yr{q}") for q in range(NPAIR)]
            cp = S.tile([128, 102], f32)

            # sync: a-side (head, queries, then later a-chunks + consts);
            # gpsimd: b-side + Y tables interleaved by deadline
            nc.sync.dma_start(out=T[0:64, 0:256], in_=tXad[:, 0:256])
            nc.gpsimd.dma_start(out=T[64:128, 0:256], in_=tXbd[:, 0:256])
            nc.sync.dma_start(out=T[0:64, 256:768], in_=tXad[:, 256:768])
            nc.gpsimd.dma_start(out=T[64:128, 256:768], in_=tXbd[:, 256:768])
            nc.sync.dma_start(out=T[0:64, 768:1280], in_=tXad[:, 768:1280])
            nc.gpsimd.dma_start(out=Yr[0], in_=Yrd[0])
            nc.gpsimd.dma_start(out=T[64:128, 768:1280], in_=tXbd[:, 768:1280])
            nc.sync.dma_start(out=T[0:64, 1280:1792], in_=tXad[:, 1280:1792])
            nc.sync.dma_start(out=cp, in_=cpd)
            nc.gpsimd.dma_start(out=T[64:128, 1280:1792], in_=tXbd[:, 1280:1792])
            nc.sync.dma_start(out=T[0:64, 1792:2304], in_=tXad[:, 1792:2304])
            nc.gpsimd.dma_start(out=Yr[1], in_=Yrd[1])
            nc.gpsimd.dma_start(out=T[64:128, 1792:2304], in_=tXbd[:, 1792:2304])
            nc.gpsimd.dma_start(out=Yr[2], in_=Yrd[2])
            nc.gpsimd.dma_start(out=Yr[3], in_=Yrd[3])

            w1a = T[0:64, 0:128]
            w1b = T[64:128, 0:128]
            w2r = T[:, 128:256]
            kmask2 = cp[:, 1:21]
            ncq = cp[:, 21:22]
            yext = cp[:, 22:102]

            pacc = S.tile([128, 10], f32)   # num accums 0:5 | den 5:10
            nparts = pacc[:, 0:5]
            dparts = pacc[:, 5:10]
            parts = S.tile([128, 2], f32)
            AA = S.tile([128, 2 * O], bf16)
            ET0 = S.tile([128, 1024], bf16)
            nall = S.tile([128, 4 * 2 * O], f32)
            rsb = S.tile([128, 4 * O], f32)
            osb = S.tile([128, 4 * O], f32)

            # ---- PE warmup: dummy matmuls during the DMA wait trigger the
            # HAM clock boost before the real pipeline starts
            wps = PS.tile([128, 1024], f32, tag="hps", bufs=2, name="wps")
            for _ in range(12):
                nc.tensor.matmul(wps[0:16, 0:16], lhsT=wmm[0:1, 0:16],
                                 rhs=wmm[0:1, 0:16], start=True, stop=True)

            # ---- 5 train stages; s0 = queries only (h0-half, starts on
            # half the data), s1-s3 = full 1024-wide pairs, s4 = h1-only 512
            # so the final serial drain into the finalize is short.
            SA = [(0, 512), (512, 1024), (1024, 1536), (1536, 2048), None]
            SB = [None, (0, 512), (512, 1024), (1024, 1536), (1536, 2048)]
            SWD = [512, 1024, 1024, 1024, 512]
            RELU_ACT = [False, True, False, False, True]
            hps_t = []

            def w1_stage(s):
                hps = PS.tile([128, 1024], f32, tag="hps", bufs=2,
                              name=f"hps{s}")
                hps_t.append(hps)
                # h0 -> cols [0:512] (bank0), h1 -> [512:1024] (bank1) so
                # the concurrent quadrant MMs hit different PSUM banks;
                # an h1-only stage writes [0:512] alone.
                if SA[s] is not None:
                    a0, a1 = SA[s]
                    nc.tensor.matmul(hps[:, 0:512], lhsT=w1a,
                                     rhs=T[0:64, 256 + a0:256 + a1],
                                     start=True, stop=True)
                if SB[s] is not None:
                    b0, b1 = SB[s]
                    dst = slice(512, 1024) if SA[s] is not None else slice(0, 512)
                    nc.tensor.matmul(hps[:, dst], lhsT=w1b,
                                     rhs=T[64:128, 256 + b0:256 + b1],
                                     start=True, stop=True)

            w1_stage(0)
            for s in range(5):
                w = SWD[s]
                h1 = W.tile([128, 1024], bf16, tag="h1", bufs=4)
                if RELU_ACT[s]:
                    nc.scalar.activation(out=h1[:, 0:w], in_=hps_t[s][:, 0:w],
                                         func=AF.Relu)
                else:
                    nc.vector.tensor_scalar(out=h1[:, 0:w],
                                            in0=hps_t[s][:, 0:w],
                                            scalar1=0.0,
                                            scalar2=None, op0=ALU.max)
                if s + 1 < 5:
                    w1_stage(s + 1)
                xr = PS.tile([128, 1024], f32, tag="xr", bufs=2, name=f"xr{s}")
                nc.tensor.matmul(xr[:, 0:512], lhsT=w2r, rhs=h1[:, 0:512],
                                 start=True, stop=True)
                if w > 512:
                    nc.tensor.matmul(xr[:, 512:1024], lhsT=w2r,
                                     rhs=h1[:, 512:1024],
                                     start=True, stop=True)
                # fused Gaussian: DErf(xr/h - c_g/h) = 2/sqrt(pi) exp(-s(xr-c)^2)
                ET = ET0 if s == 0 else W.tile([128, 1024], bf16, tag="ET",
                                               bufs=4)
                scr = W.tile([128, 1024], bf16, tag="scr", bufs=4)
                Yin = (Yr[0][:, 0:512] if s == 0 else
                       Yr[0][:, 512:1024] if s == 4 else Yr[s])
                nc.scalar.activation(out=ET[:, 0:w], in_=xr[:, 0:w],
                                     func=AF.Derivative_Erf,
                                     bias=ncq, scale=inv_h,
                                     accum_out=dparts[:, s:s + 1])
                nc.vector.scalar_tensor_tensor(
                    out=scr[:, 0:w], in0=ET[:, 0:w], scalar=1.0,
                    in1=Yin, op0=ALU.bypass, op1=ALU.mult,
                    accum_out=nparts[:, s:s + 1])

            # ---- tables -> AA (bf16 for the bf16 fin matmuls) ----
            nc.vector.tensor_reduce(
                out=parts, in_=pacc.rearrange("p (k s) -> p k s", s=5),
                axis=mybir.AxisListType.X, op=ALU.add)
            PP = parts.ap[0][0]
            parts_b = bass.AP(tensor=parts.tensor, offset=parts.offset,
                              ap=[[PP, 128], [1, 2], [0, O]])
            nc.vector.tensor_tensor(out=AA.rearrange("p (k e) -> p k e", e=O),
                                    in0=parts_b,
                                    in1=kmask2.rearrange("p (k e) -> p k e", e=O),
                                    op=ALU.mult)

            # ---- query contraction directly into query-partition layout
            fin = PS.tile([128, 4 * 2 * O], f32, tag="xr", bufs=2, name="fin")
            for j in range(4):
                nc.tensor.matmul(fin[:, j * 20:(j + 1) * 20],
                                 lhsT=ET0[:, j * 128:(j + 1) * 128], rhs=AA,
                                 start=True, stop=True)

            # single subtract: num-Y and den-1 at once (yext = Y | ones)
            nc.vector.tensor_tensor(out=nall, in0=fin, in1=yext,
                                    op=ALU.subtract)
            NP = nall.ap[0][0]
            numv = bass.AP(tensor=nall.tensor, offset=nall.offset,
                           ap=[[NP, 128], [2 * O, 4], [1, O]])
            denv = bass.AP(tensor=nall.tensor, offset=nall.offset + O,
                           ap=[[NP, 128], [2 * O, 4], [1, O]])
            nc.vector.reciprocal_approx_fast(
                rsb.rearrange("p (j o) -> p j o", o=O), denv)
            nc.vector.tensor_tensor(
                out=osb.rearrange("p (j o) -> p j o", o=O),
                in0=numv, in1=rsb.rearrange("p (j o) -> p j o", o=O),
                op=ALU.mult)
            nc.sync.dma_start(out=out, in_=osb)

    nc.compile()
    return nc


def build_in_maps(x, train_X, Y, W1, W2, h):
    import jax.numpy as jnp

    def bf(a):
        return np.asarray(jnp.asarray(np.ascontiguousarray(a),
                                      dtype=jnp.bfloat16))

    cbase = _host_consts(float(h))
    x = np.ascontiguousarray(x, dtype=np.float32)
    train_X = np.ascontiguousarray(train_X, dtype=np.float32)
    Y = np.ascontiguousarray(Y, dtype=np.float32)
    W1 = np.ascontiguousarray(W1, dtype=np.float32)
    W2 = np.ascontiguousarray(W2, dtype=np.float32)

    pmod = np.arange(128) % O
    W2rep = W2[pmod].T                  # [128k, 128p]; W2rep[k,p] = W2[p%10,k]

    idx = np.arange(N)
    in_maps = []
    for c in range(NCORES):
        n_list = (idx + c * BQ) % N     # core's own queries first
        Xp = train_X[n_list]            # [N, 64]
        Yp = Y[n_list][:, pmod]         # [N, 128]
        tXa = np.concatenate([W1.T, W2rep[0:64, :], Xp[0:2048].T], axis=1)
        tXb = np.concatenate([W1.T, W2rep[64:128, :], Xp[2048:4096].T], axis=1)
        # pair q columns: [h0 = points q*512:(q+1)*512 | h1 = 2048+same]
        cpack = np.zeros((128, 102), np.float32)
        cpack[:, 0:22] = cbase
        cpack[:, 22:102] = _host_yext(Y, c)
        m = {"tXa": bf(tXa), "tXb": bf(tXb), "cpack": cpack}
        # Yr0 = [stage0 (queries, a-pts 0:512) | stage4 (b-pts 1536:2048)];
        # Yr1..Yr3 = stage s: [a-pts s*512:(s+1)*512 | b-pts (s-1)*512:s*512]
        m["Yr0"] = bf(np.concatenate(
            [Yp[0:512].T, Yp[2048 + 1536:2048 + 2048].T], axis=1))
        for s in range(1, 4):
            m[f"Yr{s}"] = bf(np.concatenate(
                [Yp[s * 512:(s + 1) * 512].T,
                 Yp[2048 + (s - 1) * 512:2048 + s * 512].T], axis=1))
        in_maps.append(m)
    return in_maps


def kernel(x, train_X, Y, W1, W2, h):
    import concourse.bass_utils as bass_utils

    hval = float(h)
    key = ("fgt26", hval)
    if key not in _cache:
        _cache[key] = _build(hval)
    nc = _cache[key]

    in_maps = build_in_maps(x, train_X, Y, W1, W2, h)
    res = bass_utils.run_bass_kernel_spmd(nc, in_maps, core_ids=list(range(NCORES)))
    # device out is [128, 4*O] with row p, col j*O+e = query (j*128+p);
    # un-interleave on host
    outs = []
    for c in range(NCORES):
        a = np.asarray(res.results[c]["out"]).reshape(128, 4, O)
        outs.append(np.ascontiguousarray(a.transpose(1, 0, 2)).reshape(BQ, O))
    return np.concatenate(outs, axis=0)
